# revision 9
# baseline (speedup 1.0000x reference)
"""Trainium2 Bass kernel for nn_CombineRadialSpeciesWithAngular.

Per-angular-order GEMM out_l = v_l @ W[l], flattened+concatenated over l.
Full shapes: v_l [20000, 2l+1, 128] f32 (l=0..5), W [6, 128, 256] f32,
out [720000, 256] f32.

Strategy (8 NeuronCores, data-parallel over samples; ~183us HW):
  - Each core gets 2500 samples of every block -> 90000 output rows.
  - bf16 end to end on the device (rel-err budget 2e-2 >> bf16's ~3e-3):
    host pre-transposes each core's rows into vt [128, 90000] bf16
    (contraction dim on partitions), W -> [128, 6, 256] bf16 replicated;
    the device RETURNS bf16 too and the host upcasts. 23 MB in + 46 MB
    out per core (vs 138 MB all-f32) -> the kernel rides the DMA
    roofline at ~26 GB/s x 16 SDMA engines.
  - W-stationary matmuls: stationary = W[l] column-half [128, 128]
    (FWL engages at 128 cols bf16), moving = vt chunk [128, 500 rows],
    PSUM gets out^T [128 cols, 500 rows] (1 bank), so there are only
    360 big matmuls and no per-chunk weight reloads. The device output
    is out^T [2, 128, 90000] bf16; the host transposes during unshard.
  - PSUM -> SBUF bf16 casts alternate between DVE and ACT (either alone
    would pace the kernel); outputs leave via SWDGE (nc.gpsimd), which
    swizzles descriptors across all 16 SDMA engines by partition --
    HWDGE slab-splits contiguous-DRAM destinations 512KB-per-engine
    from a fixed base (the f32 baseline's real bottleneck: all output
    bytes on 5 of 16 engines). Inputs (SBUF-dst, already partition-
    swizzled) stay on HWDGE (nc.sync), 2500-row tiles so the first
    matmul starts early and prefetch stays fine-grained.

Uses bacc.Bacc (not bass.Bass): its compile pipeline legalizes semaphore
waits to this target's 1-wait-per-instruction limit; plain Bass output
fails walrus codegen ("Too many sync wait commands").
"""

import math
import sys

import numpy as np

for _p in ("/opt/trn_rl_repo", "/root/.axon_site/_ro/trn_rl_repo"):
    if _p not in sys.path:
        sys.path.append(_p)

import ml_dtypes

import concourse.bacc as bacc
import concourse.mybir as mybir
import concourse.tile as tile
from concourse.bass_utils import run_bass_kernel_spmd

N_CORES = 8
N_SAMPLES = 20000
N_PROPS = 128
N_COMB = 256
N_ANG = 6
S_CORE = N_SAMPLES // N_CORES          # 2500
M_TOTAL = sum(2 * l + 1 for l in range(N_ANG))  # 36
ROWS = S_CORE * M_TOTAL                # 90000
TILE_R = 5000                          # rows per output tile (per half)
NTILE = ROWS // TILE_R                 # 18
VT_R = 2500                            # rows per input DMA tile
CH = 500                               # rows per matmul chunk
NCH = TILE_R // CH                     # 10

F32 = mybir.dt.float32
BF16 = mybir.dt.bfloat16
BF16_NP = ml_dtypes.bfloat16

_nc_cache = {}


def build_nc(reps=1):
    if reps in _nc_cache:
        return _nc_cache[reps]

    nc = bacc.Bacc()
    vt = nc.dram_tensor("vt", [128, ROWS], BF16, kind="ExternalInput")
    w = nc.dram_tensor("w", [128, N_ANG, N_COMB], BF16, kind="ExternalInput")
    out = nc.dram_tensor("out", [2, 128, ROWS], BF16, kind="ExternalOutput")

    with tile.TileContext(nc) as tc:
        with (
            tc.tile_pool(name="wp", bufs=1) as wp,
            tc.tile_pool(name="vp", bufs=8) as vp,
            tc.tile_pool(name="op", bufs=6) as op,
            tc.tile_pool(name="pp", bufs=8, space="PSUM") as pp,
        ):
            wt = wp.tile([128, N_ANG, 2, 128], BF16)
            nc.sync.dma_start(
                wt[:], w.rearrange("p l (h c) -> p l h c", h=2))

            for rep in range(reps):
                # input stream: small tiles so the first matmul starts
                # early and prefetch stays fine-grained (2500 rows = 2
                # output tiles' worth per vt tile)
                vts = []
                for v in range(ROWS // VT_R):
                    vt_t = vp.tile([128, VT_R], BF16)
                    nc.sync.dma_start(
                        vt_t[:], vt[:, v * VT_R:(v + 1) * VT_R])
                    vts.append(vt_t)

                for t in range(NTILE):
                    r0 = t * TILE_R
                    for h in range(2):
                        ot = op.tile([128, TILE_R], BF16)
                        for c in range(NCH):
                            r = r0 + c * CH
                            l = math.isqrt(r // S_CORE)
                            vt_t = vts[r // VT_R]
                            co = r % VT_R
                            ps_t = pp.tile([128, CH], F32)
                            nc.tensor.matmul(
                                ps_t[:],
                                wt[:, l, h, :],
                                vt_t[:, co:co + CH],
                                start=True, stop=True)
                            dst = ot[:, c * CH:(c + 1) * CH]
                            if c % 2 == 0:
                                nc.vector.tensor_copy(dst, ps_t[:])
                            else:
                                nc.scalar.copy(dst, ps_t[:])
                        nc.gpsimd.dma_start(
                            out[h, :, r0:r0 + TILE_R], ot[:])

    nc.finalize()
    _nc_cache[reps] = nc
    return nc


def shard_inputs(inputs):
    """Full inputs -> per-core in_maps (host transpose + cast to bf16)."""
    w = np.ascontiguousarray(
        np.asarray(inputs["W"], dtype=np.float32).transpose(1, 0, 2)
    ).astype(BF16_NP)
    in_maps = []
    for i in range(N_CORES):
        vt_i = np.empty((128, ROWS), dtype=np.float32)
        col = 0
        for l in range(N_ANG):
            n = S_CORE * (2 * l + 1)
            blk = np.asarray(inputs[f"values_l{l}"][i * S_CORE:(i + 1) * S_CORE],
                             dtype=np.float32)
            vt_i[:, col:col + n] = blk.reshape(n, 128).T
            col += n
        in_maps.append({"vt": vt_i.astype(BF16_NP), "w": w})
    return in_maps


def unshard_output(core_outs):
    """Per-core bf16 out^T [2, 128, 90000] -> full f32 [720000, 256]."""
    full = np.empty((N_SAMPLES * M_TOTAL, N_COMB), dtype=np.float32)
    for i, o in enumerate(core_outs):
        o = np.asarray(o)                              # [2, 128, 90000]
        for l in range(N_ANG):
            n = S_CORE * (2 * l + 1)
            src0 = S_CORE * l * l
            dst0 = N_SAMPLES * l * l + i * n
            blk = o[:, :, src0:src0 + n]               # [2, 128, n]
            full[dst0:dst0 + n] = (
                blk.transpose(2, 0, 1).reshape(n, N_COMB).astype(np.float32))
    return full


def run_sharded(in_maps, **kwargs):
    nc = build_nc()
    return run_bass_kernel_spmd(nc, in_maps, core_ids=list(range(N_CORES)),
                                **kwargs)


def kernel(**inputs):
    res = run_sharded(shard_inputs(inputs))
    return unshard_output([r["out"] for r in res.results])


# revision 11
# speedup vs baseline: 1.1356x; 1.1356x over previous
"""Trainium2 Bass kernel for nn_CombineRadialSpeciesWithAngular.

Per-angular-order GEMM out_l = v_l @ W[l], flattened+concatenated over l.
Full shapes: v_l [20000, 2l+1, 128] f32 (l=0..5), W [6, 128, 256] f32,
out [720000, 256] f32.

Strategy (8 NeuronCores, data-parallel over samples; ~185-210us HW vs
975us f32 baseline):
  - Each core gets 2500 samples of every block -> 90000 output rows.
  - bf16 end to end on the device (rel-err budget 2e-2 >> bf16's ~3e-3):
    host pre-transposes each core's rows into vt [128, 90000] bf16
    (contraction dim on partitions), W -> [128, 6, 256] bf16 replicated;
    the device RETURNS bf16 too and the host upcasts. 23 MB in + 46 MB
    out per core (vs 138 MB all-f32) -> the kernel rides the DMA
    roofline at ~26 GB/s x 16 SDMA engines (~95% engine duty).
  - W-stationary matmuls: stationary = W[l] column-half [128, 128]
    (FWL engages at 128 cols bf16), moving = vt chunk [128, 500 rows],
    PSUM gets out^T [128 cols, 500 rows] (1 bank), so there are only
    360 big matmuls and no per-chunk weight reloads. The device output
    is out^T [2, 128, 90000] bf16; the host transposes during unshard.
  - PSUM -> SBUF bf16 casts alternate between DVE and ACT (either alone
    would pace the kernel); outputs leave via SWDGE (nc.gpsimd), which
    swizzles descriptors across all 16 SDMA engines by partition --
    HWDGE slab-splits contiguous-DRAM destinations 512KB-per-engine
    from a fixed base (the f32 baseline's real bottleneck: all output
    bytes on 5 of 16 engines, ~130 GB/s). Inputs (SBUF-dst, already
    partition-swizzled) stay on HWDGE (nc.sync).
  - Startup: W split into an l=0 tile (64KB, loaded first) + the rest,
    and input segments 500+2000+2500 then 17x5000 rows, so the first
    matmul starts ~10us in instead of ~18us; 5000-row steady-state
    input tiles keep per-partition DMA runs at 10KB.

Uses bacc.Bacc (not bass.Bass): its compile pipeline legalizes semaphore
waits to this target's 1-wait-per-instruction limit; plain Bass output
fails walrus codegen ("Too many sync wait commands").
"""

import bisect
import math
import sys

import numpy as np

for _p in ("/opt/trn_rl_repo", "/root/.axon_site/_ro/trn_rl_repo"):
    if _p not in sys.path:
        sys.path.append(_p)

import ml_dtypes

import concourse.bacc as bacc
import concourse.mybir as mybir
import concourse.tile as tile
from concourse.bass_utils import run_bass_kernel_spmd

N_CORES = 8
N_SAMPLES = 20000
N_PROPS = 128
N_COMB = 256
N_ANG = 6
S_CORE = N_SAMPLES // N_CORES          # 2500
M_TOTAL = sum(2 * l + 1 for l in range(N_ANG))  # 36
ROWS = S_CORE * M_TOTAL                # 90000
TILE_R = 5000                          # rows per output tile (per half)
NTILE = ROWS // TILE_R                 # 18
CH = 500                               # rows per matmul chunk
NCH = TILE_R // CH                     # 10

# input segments (all bounds multiples of CH)
SEGS = [(0, 500), (500, 2000), (2500, 2500)] + [
    (5000 + i * 5000, 5000) for i in range(17)]
SEG_STARTS = [s for s, _ in SEGS]

F32 = mybir.dt.float32
BF16 = mybir.dt.bfloat16
BF16_NP = ml_dtypes.bfloat16

_nc_cache = {}


def build_nc(reps=1):
    if reps in _nc_cache:
        return _nc_cache[reps]

    nc = bacc.Bacc()
    vt = nc.dram_tensor("vt", [128, ROWS], BF16, kind="ExternalInput")
    w = nc.dram_tensor("w", [128, N_ANG, N_COMB], BF16, kind="ExternalInput")
    out = nc.dram_tensor("out", [2, 128, ROWS], BF16, kind="ExternalOutput")
    w_v = w.rearrange("p l (h c) -> p l h c", h=2)

    with tile.TileContext(nc) as tc:
        with (
            tc.tile_pool(name="wp", bufs=1) as wp,
            tc.tile_pool(name="vp", bufs=8) as vp,
            tc.tile_pool(name="op", bufs=6) as op,
            tc.tile_pool(name="pp", bufs=8, space="PSUM") as pp,
        ):
            w0 = wp.tile([128, 2, 128], BF16)
            wr = wp.tile([128, N_ANG - 1, 2, 128], BF16)
            nc.sync.dma_start(w0[:], w_v[:, 0])

            for rep in range(reps):
                vts = []
                for si, (r0, nr) in enumerate(SEGS):
                    vt_t = vp.tile([128, nr], BF16)
                    nc.sync.dma_start(vt_t[:], vt[:, r0:r0 + nr])
                    vts.append(vt_t)
                    if si == 0 and rep == 0:
                        nc.sync.dma_start(wr[:], w_v[:, 1:])

                for t in range(NTILE):
                    r0 = t * TILE_R
                    for h in range(2):
                        ot = op.tile([128, TILE_R], BF16)
                        for c in range(NCH):
                            r = r0 + c * CH
                            l = math.isqrt(r // S_CORE)
                            si = bisect.bisect_right(SEG_STARTS, r) - 1
                            vt_t = vts[si]
                            co = r - SEGS[si][0]
                            ps_t = pp.tile([128, CH], F32)
                            nc.tensor.matmul(
                                ps_t[:],
                                w0[:, h, :] if l == 0 else wr[:, l - 1, h, :],
                                vt_t[:, co:co + CH],
                                start=True, stop=True)
                            dst = ot[:, c * CH:(c + 1) * CH]
                            if c % 2 == 0:
                                nc.vector.tensor_copy(dst, ps_t[:])
                            else:
                                nc.scalar.copy(dst, ps_t[:])
                        nc.gpsimd.dma_start(
                            out[h, :, r0:r0 + TILE_R], ot[:])

    nc.finalize()
    _nc_cache[reps] = nc
    return nc


def shard_inputs(inputs):
    """Full inputs -> per-core in_maps (host transpose + cast to bf16)."""
    w = np.ascontiguousarray(
        np.asarray(inputs["W"], dtype=np.float32).transpose(1, 0, 2)
    ).astype(BF16_NP)
    in_maps = []
    for i in range(N_CORES):
        vt_i = np.empty((128, ROWS), dtype=np.float32)
        col = 0
        for l in range(N_ANG):
            n = S_CORE * (2 * l + 1)
            blk = np.asarray(inputs[f"values_l{l}"][i * S_CORE:(i + 1) * S_CORE],
                             dtype=np.float32)
            vt_i[:, col:col + n] = blk.reshape(n, 128).T
            col += n
        in_maps.append({"vt": vt_i.astype(BF16_NP), "w": w})
    return in_maps


def unshard_output(core_outs):
    """Per-core bf16 out^T [2, 128, 90000] -> full f32 [720000, 256]."""
    full = np.empty((N_SAMPLES * M_TOTAL, N_COMB), dtype=np.float32)
    for i, o in enumerate(core_outs):
        o = np.asarray(o)                              # [2, 128, 90000]
        for l in range(N_ANG):
            n = S_CORE * (2 * l + 1)
            src0 = S_CORE * l * l
            dst0 = N_SAMPLES * l * l + i * n
            blk = o[:, :, src0:src0 + n]               # [2, 128, n]
            full[dst0:dst0 + n] = (
                blk.transpose(2, 0, 1).reshape(n, N_COMB).astype(np.float32))
    return full


def run_sharded(in_maps, **kwargs):
    nc = build_nc()
    return run_bass_kernel_spmd(nc, in_maps, core_ids=list(range(N_CORES)),
                                **kwargs)


def kernel(**inputs):
    res = run_sharded(shard_inputs(inputs))
    return unshard_output([r["out"] for r in res.results])
